# revision 22
# baseline (speedup 1.0000x reference)
"""Trainium2 Bass kernel for LogHarmonicLowering.

out[b, k*C + c, j, t] = wv0[k,j] * x[b, c, j+d_k, t] + wv1[k,j] * x[b, c, j+d_k+1, t]

with zero padding past the frequency range. The bilinear shift per k has a
constant integer part d_k plus per-(k,j) float32 weights wv0/wv1 precomputed
on host with the exact float32 arithmetic of the reference's grid method.

Distribution: data-parallel over batch — 8 cores, one batch element each.

Per-core scheme: partition dim = frequency row within a 128-row half; the
half index h lives on the free axis, so each channel-group tile is
X[p, c, h, t] = x[c, h*128+p, t] with shape [128, G, 2, T]. X1 is the same
tile shifted down one frequency row. Per (group, k):

  ACT:  Z[:, :, h] = wv1[k, j(h,p)] * X1[:, :, h]       (Copy activation,
                                                         per-partition scale)
  DVE:  Z[:, :, h] = (X[:, :, h] * wv0[k, j(h,p)]) + Z[:, :, h]
                                         (scalar_tensor_tensor, in place)

The integer shift d_k is applied via the store DMA's SBUF-side partition
offset (DMAs have no partition-alignment restriction; compute operands
must start at partition 0/32/64/96, so a partition-shifted operand read is
illegal). k = 0 (shift 0) is an exact copy: X tiles are stored straight
back. Trailing d_k zero rows come from a persistent zeroed tile.

Two ways to materialize X1:
  pe_x1=False: load it from DRAM again (re-reads x, ~17MB extra traffic)
  pe_x1=True:  PE computes it into PSUM with exact 0/1 f32 shift matrices
               (X1A = S@XA + E@XB accumulated, X1B = S@XB; S = subdiagonal,
               E picks XB row 0 into row 127); ACT then reads PSUM.
               This removes the duplicate HBM read entirely.

Hand-scheduled BSP pipeline (one sync wait per instruction). All DMA
completion semaphores are per-buffer-slot and every issuing engine waits
on its own sem before starting a new burst of increments on it (the race
detector requires burst ordering; completions across bursts are otherwise
unordered).

Engines: SP loads/k0 copies/zero stores + drain, PE the X1 shift (pe_x1),
ACT the wv1 product, DVE the fused multiply-add, gpsimd the Z stores.

`reps` repeats the whole pipeline inside one program (buffer-slot
rotation continues across repeats) — used by test.py to measure per-
execution device time differentially; the program is idempotent.
"""

import functools

import numpy as np

import concourse.bass as bass
import concourse.mybir as mybir
from concourse.bass_utils import run_bass_kernel_spmd

FK = 5
ANCHOR = 1
OUT_LOG = 12.0
IN_LOG = 1.0
RADIX = 2.0

B, C, F, T = 8, 32, 256, 512
N_CORES = 8


def _host_weights(Fr):
    """Per-(k, j) bilinear weights, float32 ops matching the jax reference."""
    np_shift = (np.arange(FK) + 1) / ANCHOR
    ls = OUT_LOG * np.log(IN_LOG * np_shift) / np.log(RADIX)
    ls -= ls[ANCHOR - 1]
    ls32 = ls.astype(np.float32)
    shift_px = ls32 * np.float32(Fr / (Fr - 1))
    y = np.arange(Fr, dtype=np.float32)[None, :] + shift_px[:, None]
    y0f = np.floor(y)
    w1 = y - y0f
    w0 = np.float32(1.0) - w1
    y0 = y0f.astype(np.int32)
    y1 = y0 + 1
    v0 = ((y0 >= 0) & (y0 < Fr)).astype(np.float32)
    v1 = ((y1 >= 0) & (y1 < Fr)).astype(np.float32)
    wv0 = w0 * v0
    wv1 = w1 * v1
    d = y0[:, 0]
    # the integer shift is constant along j (fractional parts never round
    # across an integer boundary in f32 for these shifts)
    assert (y0 == d[:, None] + np.arange(Fr, dtype=np.int32)[None, :]).all()
    return wv0, wv1, d


def build_nc(C=C, Fr=F, T=T, G=2, NBUF=4, ZBUF=6, reps=1, pe_x1=True,
             store_coarse=False, fat_zeros=False, swap_roles=False):
    """Raw-bass per-core program: x[C,Fr,T] -> out[FK*C,Fr,T]."""
    from contextlib import ExitStack

    wv0, wv1, dks = _host_weights(Fr)
    H = Fr // 2
    assert H == 128
    nG = C // G
    nGT = nG * reps
    nK = FK - 1
    f32 = mybir.dt.float32
    dmax = int(dks.max())

    # weight table: 16 columns, one per (a in {wv0, wv1}, ki in 0..3, h in 0..1)
    ncols = 2 * nK * 2

    def col(a, ki, h):
        return (a * nK + ki) * 2 + h

    wvtab = np.zeros((H, ncols), np.float32)
    p = np.arange(H)
    for ki in range(nK):
        k = ki + 1
        d = int(dks[k])
        for a, wv in ((0, wv0), (1, wv1)):
            # h=0: output row j = p - d (invalid rows weighted 0, never stored)
            wvtab[:, col(a, ki, 0)] = np.where(
                p >= d, wv[k, np.maximum(p - d, 0)], np.float32(0))
            # h=1: output row j = H + p - d (always in range)
            wvtab[:, col(a, ki, 1)] = wv[k, p + H - d]

    # PE shift matrices (lhsT layout: [K=i, M=p]): X1[p] = x[p+1]
    se = np.zeros((H, 2 * H), np.float32)
    se[np.arange(1, H), np.arange(H - 1)] = 1.0          # S: i = p+1
    se[0, H + 127] = 1.0                                 # E: row127 += XB[0]

    Copy = mybir.ActivationFunctionType.Copy
    mult = mybir.AluOpType.mult
    add = mybir.AluOpType.add

    nc = bass.Bass(trn_type="TRN2")
    x_h = nc.dram_tensor("x", [C, Fr, T], f32, kind="ExternalInput")
    out_h = nc.dram_tensor("out", [FK * C, Fr, T], f32, kind="ExternalOutput")
    wv_h = nc.inline_tensor(wvtab, name="wvtab")
    se_h = nc.inline_tensor(se, name="setab") if pe_x1 else None

    def dram2(ap):
        # [G, 2H, T] dram slice -> [p, c, h, t]
        return ap.rearrange("c (h p) t -> p c h t", h=2)

    def dram1(ap):
        # [G, rows, T] dram slice -> [p, c, 1, t]
        return ap.rearrange("c (o p) t -> p c o t", o=1)

    LD_PER_G = (1 if pe_x1 else 4) * 16
    wv_target = 32 if pe_x1 else 16

    def act_after(g, ki=nK - 1, n=2):
        return 8 * g + 2 * ki + n

    with ExitStack() as ctx:
        sb = lambda shape, name: ctx.enter_context(
            nc.sbuf_tensor(name, shape, f32))
        wvt = sb([H, ncols], "wvt")
        ZC = min(16, C) if fat_zeros else G   # channels per zero-store DMA
        zeros = sb([dmax, ZC, 1, T], "zeros")
        X = [sb([H, G, 2, T], f"X{s}") for s in range(NBUF)]
        if pe_x1:
            set_ = sb([H, 2 * H], "set")
            # two full-group PSUM slots (4 banks each at G=2), rotating on
            # group parity so PE runs a whole group ahead of ACT
            ps = [ctx.enter_context(
                nc.psum_tensor(f"ps{b}", [H, G, 2, T], f32)) for b in range(2)]
            X1 = None
        else:
            X1 = [sb([H, G, 2, T], f"X1{s}") for s in range(NBUF)]
        Z = [sb([H, G, 2, T], f"Z{s}") for s in range(ZBUF)]
        sem = lambda name: ctx.enter_context(nc.semaphore(name))
        s_wv = sem("s_wv")
        s_ld = [sem(f"s_ld{s}") for s in range(NBUF)]
        s_pe = sem("s_pe")
        s_act = sem("s_act")
        s_dve = sem("s_dve")
        s_stk = [sem(f"s_stk{s}") for s in range(2)]   # k0 copy stores
        s_st = [sem(f"s_st{z}") for z in range(ZBUF)]  # Z stores
        s_z = sem("s_z")       # memset + zero-row stores
        block = ctx.enter_context(nc.Block())

        class W:  # monotone wait elision per engine
            def __init__(self, e):
                self.e, self.seen = e, {}

            def __call__(self, sem_, v):
                if v > self.seen.get(id(sem_), 0):
                    self.e.wait_ge(sem_, v)
                    self.seen[id(sem_)] = v

        @block.sync
        def _(e):
            w = W(e)
            e.dma_start(out=wvt[:, :], in_=wv_h[:, :]).then_inc(s_wv, 16)
            if pe_x1:
                e.dma_start(out=set_[:, :], in_=se_h[:, :]).then_inc(s_wv, 16)

            def issue_loads(gg):
                g = gg % nG
                s = gg % NBUF
                u = gg // NBUF
                w(s_ld[s], LD_PER_G * u)   # own-sem burst order for detector
                xg = x_h[g * G:(g + 1) * G, :, :]
                e.dma_start(out=X[s][:, :, :, :],
                            in_=dram2(xg)).then_inc(s_ld[s], 16)
                if not pe_x1:
                    e.dma_start(out=X1[s][:, :, 0:1, :],
                                in_=dram1(xg[:, 1:H + 1, :])
                                ).then_inc(s_ld[s], 16)
                    e.dma_start(out=X1[s][0:H - 1, :, 1:2, :],
                                in_=dram1(xg[:, H + 1:Fr, :])
                                ).then_inc(s_ld[s], 16)
                    e.dma_start(out=X1[s][H - 1:H, :, 1:2, :],
                                in_=dram1(xg[:, Fr - 1:Fr, :])
                                ).then_inc(s_ld[s], 16)

            for gg in range(min(NBUF, nGT)):
                issue_loads(gg)
            for gg in range(nGT):
                g = gg % nG
                s = gg % NBUF
                # k0 straight copy (needs group gg loads complete)
                og0 = out_h[g * G:(g + 1) * G, :, :]
                w(s_ld[s], LD_PER_G * (gg // NBUF + 1))
                w(s_stk[gg % 2], 16 * (gg // 2))   # own-sem burst order
                e.dma_start(out=dram2(og0), in_=X[s][:, :, :, :]
                            ).then_inc(s_stk[gg % 2], 16)
                if not fat_zeros:
                    # zero-row tails for each k (independent of compute)
                    w(s_z, 1 + 4 * 16 * gg)        # own-sem burst order
                    for ki in range(nK):
                        k = ki + 1
                        d = int(dks[k])
                        og = out_h[k * C + g * G:k * C + (g + 1) * G, :, :]
                        e.dma_start(out=dram1(og[:, Fr - d:Fr, :]),
                                    in_=zeros[0:d, :, :, :]).then_inc(s_z, 16)
                elif gg == 0:
                    # all zero-row tails up front: nK x (C/ZC) wide DMAs per
                    # rep, re-issued once per rep after the prior rep's
                    # burst completes (idempotent rewrites must be ordered)
                    r = 0
                    nzb = nK * (C // ZC)
                    w(s_z, 1 + 16 * nzb * r)
                    for ki in range(nK):
                        k = ki + 1
                        d = int(dks[k])
                        for cb in range(C // ZC):
                            og = out_h[k * C + cb * ZC:k * C + (cb + 1) * ZC,
                                       :, :]
                            e.dma_start(out=dram1(og[:, Fr - d:Fr, :]),
                                        in_=zeros[0:d, :, :, :]
                                        ).then_inc(s_z, 16)
                elif gg % nG == 0:
                    r = gg // nG
                    nzb = nK * (C // ZC)
                    w(s_z, 1 + 16 * nzb * r)
                    for ki in range(nK):
                        k = ki + 1
                        d = int(dks[k])
                        for cb in range(C // ZC):
                            og = out_h[k * C + cb * ZC:k * C + (cb + 1) * ZC,
                                       :, :]
                            e.dma_start(out=dram1(og[:, Fr - d:Fr, :]),
                                        in_=zeros[0:d, :, :, :]
                                        ).then_inc(s_z, 16)
                gn = gg + NBUF
                if gn < nGT:
                    # recycle slot: group gg fully consumed
                    if pe_x1:
                        w(s_pe, 3 * G * (gg + 1))      # PE rhs reads done
                    else:
                        w(s_act, act_after(gg))        # X1 reads done
                    if swap_roles:
                        w(s_act, act_after(gg))        # X reads done (ACT)
                    else:
                        w(s_dve, act_after(gg))        # X reads done (stt)
                    w(s_stk[gg % 2], 16 * (gg // 2 + 1))  # k0 store done
                    issue_loads(gn)
            # drain every DMA sem before program end
            for z in range(ZBUF):
                uses = sum(1 for i in range(nK * nGT) if i % ZBUF == z)
                w(s_st[z], 32 * uses)
            for par in range(2):
                uses = sum(1 for gg in range(nGT) if gg % 2 == par)
                w(s_stk[par], 16 * uses)
            if fat_zeros:
                w(s_z, 1 + 16 * nK * (C // ZC) * reps)
            else:
                w(s_z, 1 + 4 * 16 * nGT)
            for s in range(NBUF):
                uses = sum(1 for gg in range(nGT) if gg % NBUF == s)
                w(s_ld[s], LD_PER_G * uses)

        if pe_x1:
            # per group (slot b = gg%2): for each c the same-bank group
            # [S@XA_c (start), E@XB_c (accum+stop)] -> ps[b] h0 bank, then
            # the standalone [S@XB_c] -> ps[b] h1 bank. PE runs one full
            # group ahead of ACT on the other slot.
            @block.tensor
            def _(e):
                w = W(e)
                w(s_wv, wv_target)
                S_ap = set_[:, 0:H]
                E_ap = set_[:, H:2 * H]
                for gg in range(nGT):
                    s = gg % NBUF
                    b = gg % 2
                    w(s_ld[s], LD_PER_G * (gg // NBUF + 1))
                    if gg >= 2:   # ps slot recycle: reader of gg-2 done
                        w(s_dve if swap_roles else s_act, 8 * (gg - 1))
                    for c in range(G):
                        e.matmul(ps[b][:, c, 0, :], S_ap, X[s][:, c, 0, :],
                                 start=True, stop=False).then_inc(s_pe, 1)
                        e.matmul(ps[b][:, c, 0, :], E_ap, X[s][:, c, 1, :],
                                 start=False, stop=True).then_inc(s_pe, 1)
                        e.matmul(ps[b][:, c, 1, :], S_ap, X[s][:, c, 1, :],
                                 start=True, stop=True).then_inc(s_pe, 1)

        @block.scalar
        def _(e):
            w = W(e)
            w(s_wv, wv_target)
            for gg in range(nGT):
                s = gg % NBUF
                b = gg % 2
                if swap_roles:
                    w(s_ld[s], LD_PER_G * (gg // NBUF + 1))
                    src, a = X[s], 0
                elif pe_x1:
                    w(s_pe, 3 * G * (gg + 1))
                    src, a = ps[b], 1
                else:
                    w(s_ld[s], LD_PER_G * (gg // NBUF + 1))
                    src, a = X1[s], 1
                for ki in range(nK):
                    i = 4 * gg + ki
                    z = i % ZBUF
                    if i >= ZBUF:   # Z slot recycle: prior stores done
                        w(s_st[z], 32 * (i // ZBUF))
                    e.activation(Z[z][:, :, 0:1, :], src[:, :, 0:1, :], Copy,
                                 scale=wvt[:, col(a, ki, 0):col(a, ki, 0) + 1]
                                 ).then_inc(s_act, 1)
                    e.activation(Z[z][:, :, 1:2, :], src[:, :, 1:2, :], Copy,
                                 scale=wvt[:, col(a, ki, 1):col(a, ki, 1) + 1]
                                 ).then_inc(s_act, 1)

        @block.vector
        def _(e):
            w = W(e)
            e.memset(zeros[:, :, :, :], 0.0).then_inc(s_z, 1)
            w(s_wv, wv_target)
            for gg in range(nGT):
                s = gg % NBUF
                b = gg % 2
                if swap_roles:
                    w(s_pe, 3 * G * (gg + 1))
                    in0, a = ps[b], 1
                else:
                    in0, a = X[s], 0
                for ki in range(nK):
                    i = 4 * gg + ki
                    z = i % ZBUF
                    w(s_act, act_after(gg, ki, 1))
                    e.scalar_tensor_tensor(
                        Z[z][:, :, 0:1, :], in0[:, :, 0:1, :],
                        wvt[:, col(a, ki, 0):col(a, ki, 0) + 1],
                        Z[z][:, :, 0:1, :], mult, add).then_inc(s_dve, 1)
                    w(s_act, act_after(gg, ki, 2))
                    e.scalar_tensor_tensor(
                        Z[z][:, :, 1:2, :], in0[:, :, 1:2, :],
                        wvt[:, col(a, ki, 1):col(a, ki, 1) + 1],
                        Z[z][:, :, 1:2, :], mult, add).then_inc(s_dve, 1)

        @block.gpsimd
        def _(e):
            w = W(e)
            for gg in range(nGT):
                g = gg % nG
                if store_coarse:
                    # one wait for the whole group's DVE output, then burst
                    # all 8 store generations back-to-back (keeps the SWDGE
                    # descriptor pipeline hot)
                    w(s_dve, 8 * (gg + 1))
                for ki in range(nK):
                    k = ki + 1
                    d = int(dks[k])
                    i = 4 * gg + ki
                    z = i % ZBUF
                    og = out_h[k * C + g * G:k * C + (g + 1) * G, :, :]
                    w(s_st[z], 32 * (i // ZBUF))   # own-sem burst order
                    if not store_coarse:
                        w(s_dve, 2 * i + 1)
                    e.dma_start(out=dram1(og[:, 0:H - d, :]),
                                in_=Z[z][d:H, :, 0:1, :]).then_inc(s_st[z], 16)
                    if not store_coarse:
                        w(s_dve, 2 * i + 2)
                    e.dma_start(out=dram1(og[:, H - d:Fr - d, :]),
                                in_=Z[z][:, :, 1:2, :]).then_inc(s_st[z], 16)
    return nc


@functools.lru_cache(maxsize=1)
def _get_nc():
    return build_nc()


def _run(x, trace=False):
    in_maps = [{"x": np.ascontiguousarray(x[b])} for b in range(B)]
    res = run_bass_kernel_spmd(_get_nc(), in_maps, core_ids=list(range(N_CORES)),
                               trace=trace)
    out = np.stack([r["out"] for r in res.results], axis=0)
    return out, res


def kernel(x):
    x = np.asarray(x)
    assert x.shape == (B, C, F, T), x.shape
    out, _ = _run(x)
    return out


# revision 23
# speedup vs baseline: 1.0168x; 1.0168x over previous
"""Trainium2 Bass kernel for LogHarmonicLowering.

out[b, k*C + c, j, t] = wv0[k,j] * x[b, c, j+d_k, t] + wv1[k,j] * x[b, c, j+d_k+1, t]

with zero padding past the frequency range. The bilinear shift per k has a
constant integer part d_k plus per-(k,j) float32 weights wv0/wv1 precomputed
on host with the exact float32 arithmetic of the reference's grid method.

Distribution: data-parallel over batch — 8 cores, one batch element each.

Per-core scheme: partition dim = frequency row within a 128-row half; the
half index h lives on the free axis, so each channel-group tile is
X[p, c, h, t] = x[c, h*128+p, t] with shape [128, G, 2, T]. X1 is the same
tile shifted down one frequency row. Per (group, k):

  ACT:  Z[:, :, h] = wv1[k, j(h,p)] * X1[:, :, h]       (Copy activation,
                                                         per-partition scale)
  DVE:  Z[:, :, h] = (X[:, :, h] * wv0[k, j(h,p)]) + Z[:, :, h]
                                         (scalar_tensor_tensor, in place)

The integer shift d_k is applied via the store DMA's SBUF-side partition
offset (DMAs have no partition-alignment restriction; compute operands
must start at partition 0/32/64/96, so a partition-shifted operand read is
illegal). k = 0 (shift 0) is an exact copy: X tiles are stored straight
back. Trailing d_k zero rows come from a persistent zeroed tile.

Two ways to materialize X1:
  pe_x1=False: load it from DRAM again (re-reads x, ~17MB extra traffic)
  pe_x1=True:  PE computes it into PSUM with exact 0/1 f32 shift matrices
               (X1A = S@XA + E@XB accumulated, X1B = S@XB; S = subdiagonal,
               E picks XB row 0 into row 127); ACT then reads PSUM.
               This removes the duplicate HBM read entirely.

Hand-scheduled BSP pipeline (one sync wait per instruction). All DMA
completion semaphores are per-buffer-slot and every issuing engine waits
on its own sem before starting a new burst of increments on it (the race
detector requires burst ordering; completions across bursts are otherwise
unordered).

Engines: SP loads/k0 copies/zero stores + drain, PE the X1 shift (pe_x1),
ACT the wv1 product, DVE the fused multiply-add, gpsimd the Z stores.

`reps` repeats the whole pipeline inside one program (buffer-slot
rotation continues across repeats) — used by test.py to measure per-
execution device time differentially; the program is idempotent.
"""

import functools

import numpy as np

import concourse.bass as bass
import concourse.mybir as mybir
from concourse.bass_utils import run_bass_kernel_spmd

FK = 5
ANCHOR = 1
OUT_LOG = 12.0
IN_LOG = 1.0
RADIX = 2.0

B, C, F, T = 8, 32, 256, 512
N_CORES = 8


def _host_weights(Fr):
    """Per-(k, j) bilinear weights, float32 ops matching the jax reference."""
    np_shift = (np.arange(FK) + 1) / ANCHOR
    ls = OUT_LOG * np.log(IN_LOG * np_shift) / np.log(RADIX)
    ls -= ls[ANCHOR - 1]
    ls32 = ls.astype(np.float32)
    shift_px = ls32 * np.float32(Fr / (Fr - 1))
    y = np.arange(Fr, dtype=np.float32)[None, :] + shift_px[:, None]
    y0f = np.floor(y)
    w1 = y - y0f
    w0 = np.float32(1.0) - w1
    y0 = y0f.astype(np.int32)
    y1 = y0 + 1
    v0 = ((y0 >= 0) & (y0 < Fr)).astype(np.float32)
    v1 = ((y1 >= 0) & (y1 < Fr)).astype(np.float32)
    wv0 = w0 * v0
    wv1 = w1 * v1
    d = y0[:, 0]
    # the integer shift is constant along j (fractional parts never round
    # across an integer boundary in f32 for these shifts)
    assert (y0 == d[:, None] + np.arange(Fr, dtype=np.int32)[None, :]).all()
    return wv0, wv1, d


def build_nc(C=C, Fr=F, T=T, G=2, NBUF=6, ZBUF=6, reps=1, pe_x1=True,
             store_coarse=False, fat_zeros=False, swap_roles=False):
    """Raw-bass per-core program: x[C,Fr,T] -> out[FK*C,Fr,T]."""
    from contextlib import ExitStack

    wv0, wv1, dks = _host_weights(Fr)
    H = Fr // 2
    assert H == 128
    nG = C // G
    nGT = nG * reps
    nK = FK - 1
    f32 = mybir.dt.float32
    dmax = int(dks.max())

    # weight table: 16 columns, one per (a in {wv0, wv1}, ki in 0..3, h in 0..1)
    ncols = 2 * nK * 2

    def col(a, ki, h):
        return (a * nK + ki) * 2 + h

    wvtab = np.zeros((H, ncols), np.float32)
    p = np.arange(H)
    for ki in range(nK):
        k = ki + 1
        d = int(dks[k])
        for a, wv in ((0, wv0), (1, wv1)):
            # h=0: output row j = p - d (invalid rows weighted 0, never stored)
            wvtab[:, col(a, ki, 0)] = np.where(
                p >= d, wv[k, np.maximum(p - d, 0)], np.float32(0))
            # h=1: output row j = H + p - d (always in range)
            wvtab[:, col(a, ki, 1)] = wv[k, p + H - d]

    # PE shift matrices (lhsT layout: [K=i, M=p]): X1[p] = x[p+1]
    se = np.zeros((H, 2 * H), np.float32)
    se[np.arange(1, H), np.arange(H - 1)] = 1.0          # S: i = p+1
    se[0, H + 127] = 1.0                                 # E: row127 += XB[0]

    Copy = mybir.ActivationFunctionType.Copy
    mult = mybir.AluOpType.mult
    add = mybir.AluOpType.add

    nc = bass.Bass(trn_type="TRN2")
    x_h = nc.dram_tensor("x", [C, Fr, T], f32, kind="ExternalInput")
    out_h = nc.dram_tensor("out", [FK * C, Fr, T], f32, kind="ExternalOutput")
    wv_h = nc.inline_tensor(wvtab, name="wvtab")
    se_h = nc.inline_tensor(se, name="setab") if pe_x1 else None

    def dram2(ap):
        # [G, 2H, T] dram slice -> [p, c, h, t]
        return ap.rearrange("c (h p) t -> p c h t", h=2)

    def dram1(ap):
        # [G, rows, T] dram slice -> [p, c, 1, t]
        return ap.rearrange("c (o p) t -> p c o t", o=1)

    LD_PER_G = (1 if pe_x1 else 4) * 16
    wv_target = 32 if pe_x1 else 16

    def act_after(g, ki=nK - 1, n=2):
        return 8 * g + 2 * ki + n

    with ExitStack() as ctx:
        sb = lambda shape, name: ctx.enter_context(
            nc.sbuf_tensor(name, shape, f32))
        wvt = sb([H, ncols], "wvt")
        ZC = min(16, C) if fat_zeros else G   # channels per zero-store DMA
        zeros = sb([dmax, ZC, 1, T], "zeros")
        X = [sb([H, G, 2, T], f"X{s}") for s in range(NBUF)]
        if pe_x1:
            set_ = sb([H, 2 * H], "set")
            # two full-group PSUM slots (4 banks each at G=2), rotating on
            # group parity so PE runs a whole group ahead of ACT
            ps = [ctx.enter_context(
                nc.psum_tensor(f"ps{b}", [H, G, 2, T], f32)) for b in range(2)]
            X1 = None
        else:
            X1 = [sb([H, G, 2, T], f"X1{s}") for s in range(NBUF)]
        Z = [sb([H, G, 2, T], f"Z{s}") for s in range(ZBUF)]
        sem = lambda name: ctx.enter_context(nc.semaphore(name))
        s_wv = sem("s_wv")
        s_ld = [sem(f"s_ld{s}") for s in range(NBUF)]
        s_pe = sem("s_pe")
        s_act = sem("s_act")
        s_dve = sem("s_dve")
        s_stk = [sem(f"s_stk{s}") for s in range(2)]   # k0 copy stores
        s_st = [sem(f"s_st{z}") for z in range(ZBUF)]  # Z stores
        s_z = sem("s_z")       # memset + zero-row stores
        block = ctx.enter_context(nc.Block())

        class W:  # monotone wait elision per engine
            def __init__(self, e):
                self.e, self.seen = e, {}

            def __call__(self, sem_, v):
                if v > self.seen.get(id(sem_), 0):
                    self.e.wait_ge(sem_, v)
                    self.seen[id(sem_)] = v

        @block.sync
        def _(e):
            w = W(e)
            e.dma_start(out=wvt[:, :], in_=wv_h[:, :]).then_inc(s_wv, 16)
            if pe_x1:
                e.dma_start(out=set_[:, :], in_=se_h[:, :]).then_inc(s_wv, 16)

            def issue_loads(gg):
                g = gg % nG
                s = gg % NBUF
                u = gg // NBUF
                w(s_ld[s], LD_PER_G * u)   # own-sem burst order for detector
                xg = x_h[g * G:(g + 1) * G, :, :]
                e.dma_start(out=X[s][:, :, :, :],
                            in_=dram2(xg)).then_inc(s_ld[s], 16)
                if not pe_x1:
                    e.dma_start(out=X1[s][:, :, 0:1, :],
                                in_=dram1(xg[:, 1:H + 1, :])
                                ).then_inc(s_ld[s], 16)
                    e.dma_start(out=X1[s][0:H - 1, :, 1:2, :],
                                in_=dram1(xg[:, H + 1:Fr, :])
                                ).then_inc(s_ld[s], 16)
                    e.dma_start(out=X1[s][H - 1:H, :, 1:2, :],
                                in_=dram1(xg[:, Fr - 1:Fr, :])
                                ).then_inc(s_ld[s], 16)

            for gg in range(min(NBUF, nGT)):
                issue_loads(gg)
            for gg in range(nGT):
                g = gg % nG
                s = gg % NBUF
                # k0 straight copy (needs group gg loads complete)
                og0 = out_h[g * G:(g + 1) * G, :, :]
                w(s_ld[s], LD_PER_G * (gg // NBUF + 1))
                w(s_stk[gg % 2], 16 * (gg // 2))   # own-sem burst order
                e.dma_start(out=dram2(og0), in_=X[s][:, :, :, :]
                            ).then_inc(s_stk[gg % 2], 16)
                if not fat_zeros:
                    # zero-row tails for each k (independent of compute)
                    w(s_z, 1 + 4 * 16 * gg)        # own-sem burst order
                    for ki in range(nK):
                        k = ki + 1
                        d = int(dks[k])
                        og = out_h[k * C + g * G:k * C + (g + 1) * G, :, :]
                        e.dma_start(out=dram1(og[:, Fr - d:Fr, :]),
                                    in_=zeros[0:d, :, :, :]).then_inc(s_z, 16)
                elif gg == 0:
                    # all zero-row tails up front: nK x (C/ZC) wide DMAs per
                    # rep, re-issued once per rep after the prior rep's
                    # burst completes (idempotent rewrites must be ordered)
                    r = 0
                    nzb = nK * (C // ZC)
                    w(s_z, 1 + 16 * nzb * r)
                    for ki in range(nK):
                        k = ki + 1
                        d = int(dks[k])
                        for cb in range(C // ZC):
                            og = out_h[k * C + cb * ZC:k * C + (cb + 1) * ZC,
                                       :, :]
                            e.dma_start(out=dram1(og[:, Fr - d:Fr, :]),
                                        in_=zeros[0:d, :, :, :]
                                        ).then_inc(s_z, 16)
                elif gg % nG == 0:
                    r = gg // nG
                    nzb = nK * (C // ZC)
                    w(s_z, 1 + 16 * nzb * r)
                    for ki in range(nK):
                        k = ki + 1
                        d = int(dks[k])
                        for cb in range(C // ZC):
                            og = out_h[k * C + cb * ZC:k * C + (cb + 1) * ZC,
                                       :, :]
                            e.dma_start(out=dram1(og[:, Fr - d:Fr, :]),
                                        in_=zeros[0:d, :, :, :]
                                        ).then_inc(s_z, 16)
                gn = gg + NBUF
                if gn < nGT:
                    # recycle slot: group gg fully consumed
                    if pe_x1:
                        w(s_pe, 3 * G * (gg + 1))      # PE rhs reads done
                    else:
                        w(s_act, act_after(gg))        # X1 reads done
                    if swap_roles:
                        w(s_act, act_after(gg))        # X reads done (ACT)
                    else:
                        w(s_dve, act_after(gg))        # X reads done (stt)
                    w(s_stk[gg % 2], 16 * (gg // 2 + 1))  # k0 store done
                    issue_loads(gn)
            # drain every DMA sem before program end
            for z in range(ZBUF):
                uses = sum(1 for i in range(nK * nGT) if i % ZBUF == z)
                w(s_st[z], 32 * uses)
            for par in range(2):
                uses = sum(1 for gg in range(nGT) if gg % 2 == par)
                w(s_stk[par], 16 * uses)
            if fat_zeros:
                w(s_z, 1 + 16 * nK * (C // ZC) * reps)
            else:
                w(s_z, 1 + 4 * 16 * nGT)
            for s in range(NBUF):
                uses = sum(1 for gg in range(nGT) if gg % NBUF == s)
                w(s_ld[s], LD_PER_G * uses)

        if pe_x1:
            # per group (slot b = gg%2): for each c the same-bank group
            # [S@XA_c (start), E@XB_c (accum+stop)] -> ps[b] h0 bank, then
            # the standalone [S@XB_c] -> ps[b] h1 bank. PE runs one full
            # group ahead of ACT on the other slot.
            @block.tensor
            def _(e):
                w = W(e)
                w(s_wv, wv_target)
                S_ap = set_[:, 0:H]
                E_ap = set_[:, H:2 * H]
                for gg in range(nGT):
                    s = gg % NBUF
                    b = gg % 2
                    w(s_ld[s], LD_PER_G * (gg // NBUF + 1))
                    if gg >= 2:   # ps slot recycle: reader of gg-2 done
                        w(s_dve if swap_roles else s_act, 8 * (gg - 1))
                    for c in range(G):
                        e.matmul(ps[b][:, c, 0, :], S_ap, X[s][:, c, 0, :],
                                 start=True, stop=False).then_inc(s_pe, 1)
                        e.matmul(ps[b][:, c, 0, :], E_ap, X[s][:, c, 1, :],
                                 start=False, stop=True).then_inc(s_pe, 1)
                        e.matmul(ps[b][:, c, 1, :], S_ap, X[s][:, c, 1, :],
                                 start=True, stop=True).then_inc(s_pe, 1)

        @block.scalar
        def _(e):
            w = W(e)
            w(s_wv, wv_target)
            for gg in range(nGT):
                s = gg % NBUF
                b = gg % 2
                if swap_roles:
                    w(s_ld[s], LD_PER_G * (gg // NBUF + 1))
                    src, a = X[s], 0
                elif pe_x1:
                    w(s_pe, 3 * G * (gg + 1))
                    src, a = ps[b], 1
                else:
                    w(s_ld[s], LD_PER_G * (gg // NBUF + 1))
                    src, a = X1[s], 1
                for ki in range(nK):
                    i = 4 * gg + ki
                    z = i % ZBUF
                    if i >= ZBUF:   # Z slot recycle: prior stores done
                        w(s_st[z], 32 * (i // ZBUF))
                    e.activation(Z[z][:, :, 0:1, :], src[:, :, 0:1, :], Copy,
                                 scale=wvt[:, col(a, ki, 0):col(a, ki, 0) + 1]
                                 ).then_inc(s_act, 1)
                    e.activation(Z[z][:, :, 1:2, :], src[:, :, 1:2, :], Copy,
                                 scale=wvt[:, col(a, ki, 1):col(a, ki, 1) + 1]
                                 ).then_inc(s_act, 1)

        @block.vector
        def _(e):
            w = W(e)
            e.memset(zeros[:, :, :, :], 0.0).then_inc(s_z, 1)
            w(s_wv, wv_target)
            for gg in range(nGT):
                s = gg % NBUF
                b = gg % 2
                if swap_roles:
                    w(s_pe, 3 * G * (gg + 1))
                    in0, a = ps[b], 1
                else:
                    in0, a = X[s], 0
                for ki in range(nK):
                    i = 4 * gg + ki
                    z = i % ZBUF
                    w(s_act, act_after(gg, ki, 1))
                    e.scalar_tensor_tensor(
                        Z[z][:, :, 0:1, :], in0[:, :, 0:1, :],
                        wvt[:, col(a, ki, 0):col(a, ki, 0) + 1],
                        Z[z][:, :, 0:1, :], mult, add).then_inc(s_dve, 1)
                    w(s_act, act_after(gg, ki, 2))
                    e.scalar_tensor_tensor(
                        Z[z][:, :, 1:2, :], in0[:, :, 1:2, :],
                        wvt[:, col(a, ki, 1):col(a, ki, 1) + 1],
                        Z[z][:, :, 1:2, :], mult, add).then_inc(s_dve, 1)

        @block.gpsimd
        def _(e):
            w = W(e)
            for gg in range(nGT):
                g = gg % nG
                if store_coarse:
                    # one wait for the whole group's DVE output, then burst
                    # all 8 store generations back-to-back (keeps the SWDGE
                    # descriptor pipeline hot)
                    w(s_dve, 8 * (gg + 1))
                for ki in range(nK):
                    k = ki + 1
                    d = int(dks[k])
                    i = 4 * gg + ki
                    z = i % ZBUF
                    og = out_h[k * C + g * G:k * C + (g + 1) * G, :, :]
                    w(s_st[z], 32 * (i // ZBUF))   # own-sem burst order
                    if not store_coarse:
                        w(s_dve, 2 * i + 1)
                    e.dma_start(out=dram1(og[:, 0:H - d, :]),
                                in_=Z[z][d:H, :, 0:1, :]).then_inc(s_st[z], 16)
                    if not store_coarse:
                        w(s_dve, 2 * i + 2)
                    e.dma_start(out=dram1(og[:, H - d:Fr - d, :]),
                                in_=Z[z][:, :, 1:2, :]).then_inc(s_st[z], 16)
    return nc


@functools.lru_cache(maxsize=1)
def _get_nc():
    return build_nc()


def _run(x, trace=False):
    in_maps = [{"x": np.ascontiguousarray(x[b])} for b in range(B)]
    res = run_bass_kernel_spmd(_get_nc(), in_maps, core_ids=list(range(N_CORES)),
                               trace=trace)
    out = np.stack([r["out"] for r in res.results], axis=0)
    return out, res


def kernel(x):
    x = np.asarray(x)
    assert x.shape == (B, C, F, T), x.shape
    out, _ = _run(x)
    return out
